# revision 11
# baseline (speedup 1.0000x reference)
"""Trainium2 Bass/Tile kernel for the MipRayMarcher nn.Module.

Full inputs:
    colors         [4, 16384, 128, 3] f32
    density_logits [4, 16384, 128, 1] f32
    depths         [4, 16384, 128, 1] f32  (sorted along samples)

Returns the reference tuple:
    (composite_rgb [B,R,3], composite_depth [B,R,1], weights [B,R,127,1],
     weights_all (= weights), alpha [B,R,127,1], depths_mid [B,R,127,1])

Strategy: flatten (B, R) -> 65536 rays, shard 8192 contiguous rays per
NeuronCore (data parallel).  Per core, rays map ray-major into SBUF tiles of
[128 partitions x NR=8 ray slots x 128 samples], and every elementwise op
runs on a single flat free dimension (full engine rate).  Positions at ray
boundaries (sample 127 of each slot) carry garbage that is either never
read, excluded by strided output DMAs, or patched by small fixup copies.

The per-ray transmittance cumprod runs as ONE segmented DVE
tensor_tensor_scan per tile: state = d0[t]*state + d1[t], where d1 is a
constant reset pattern (1 at each ray's sample 0) and d0 carries
(1-alpha+EPS) shifted by one sample with 0 at ray starts.  weight_total
comes from the scan's per-ray end state by telescoping
(wt = 1 - T[127] + O(1e-8)), eliminating a full reduction.

Engine placement (measured costs): GPSIMD takes the five flat shifted
adds/products; the vector engine takes the scan, the strided color
products, all reductions, and small exact min/max clips (as 1-port
tensor_tensor ops against broadcast constants, which never contend with
GPSIMD for the shared SBUF port); ScalarE takes all transcendentals off a
single activation table set.  Work is emitted as a 3-stage software
pipeline (s1 two tiles ahead, s2 one tile ahead, s3 current) so the
in-order engine queues always hold ready work instead of head-of-line
blocking on cross-engine dependencies.
"""

import numpy as np

B, R, S = 4, 16384, 128
SM1 = S - 1
N_CORES = 8
G = B * R                      # 65536 rays
PER_CORE = G // N_CORES        # 8192 rays per core
P = 128                        # SBUF partitions
NR = 8                         # ray slots per partition per tile
TILE_RAYS = P * NR             # 1024 rays per tile
EPS = 1e-10
MIN_DEPTH, MAX_DEPTH = 0.1, 100.0

_PROGRAMS = {}


def build_program(per_core=PER_CORE, nr=NR):
    from contextlib import ExitStack

    import concourse.bacc as bacc
    import concourse.mybir as mybir
    import concourse.tile as tile
    from concourse.hw_specs import get_activation_tables

    AF = mybir.ActivationFunctionType
    ALU = mybir.AluOpType
    f32 = mybir.dt.float32

    # Keep Exp and Ln resolvable only from the combined table set, so the
    # whole kernel needs exactly one ACT table load (the selection pass is
    # greedy-first otherwise and would thrash between sets).
    tabs = get_activation_tables("gen3")
    for name, funcs in tabs.items():
        if name != "natural_log_exp_and_others":
            funcs.discard(AF.Exp)
            funcs.discard(AF.Ln)

    n_tiles = per_core // (P * nr)
    assert n_tiles * P * nr == per_core
    FS = nr * S          # flat free elems per dl/depth tile (1024)
    FSm = FS - 1         # flat shifted-op length (1023)
    FC = nr * S * 3      # colors tile free elems

    nc = bacc.Bacc("TRN2", target_bir_lowering=False, debug=False)

    def register_const(v):
        t = nc.alloc_sbuf_tensor(f"constf32-{v}", [P, 1], f32)
        nc.gpsimd.memset(t.ap(), v)
        nc.const_aps.aps[(f32, v)] = t.ap()

    for v in (-1.0, EPS, 2.0 * EPS, MIN_DEPTH, MAX_DEPTH):
        register_const(v)

    # Constant scan-reset pattern: 1.0 at each ray slot's sample 0, else 0.
    reset_t = nc.alloc_sbuf_tensor("scan_reset", [P, FS], f32)
    reset_ap = reset_t.ap()
    nc.gpsimd.memset(reset_ap, 0.0)
    nc.all_engine_barrier()
    nc.gpsimd.memset(reset_ap.rearrange("p (j s) -> p j s", j=nr)[:, :, 0:1], 1.0)
    nc.all_engine_barrier()

    clip_lo = nc.const_aps.tensor(MIN_DEPTH, (P, nr))
    clip_hi = nc.const_aps.tensor(MAX_DEPTH, (P, nr))

    colors_in = nc.dram_tensor("colors", [per_core, S, 3], f32, kind="ExternalInput").ap()
    dl_in = nc.dram_tensor("density_logits", [per_core, S], f32, kind="ExternalInput").ap()
    d_in = nc.dram_tensor("depths", [per_core, S], f32, kind="ExternalInput").ap()
    w_out = nc.dram_tensor("weights", [per_core, SM1], f32, kind="ExternalOutput").ap()
    a_out = nc.dram_tensor("alpha", [per_core, SM1], f32, kind="ExternalOutput").ap()
    dm_out = nc.dram_tensor("depths_mid", [per_core, SM1], f32, kind="ExternalOutput").ap()
    rgb_out = nc.dram_tensor("rgb", [per_core, 3], f32, kind="ExternalOutput").ap()
    dep_out = nc.dram_tensor("depth", [per_core, 1], f32, kind="ExternalOutput").ap()

    cv = colors_in.rearrange("(n p j) s c -> n p (j s c)", p=P, j=nr)
    dlv = dl_in.rearrange("(n p j) s -> n p (j s)", p=P, j=nr)
    dv = d_in.rearrange("(n p j) s -> n p (j s)", p=P, j=nr)
    wv = w_out.rearrange("(n p j) s -> n p (j s)", p=P, j=nr)
    av = a_out.rearrange("(n p j) s -> n p (j s)", p=P, j=nr)
    dmv = dm_out.rearrange("(n p j) s -> n p (j s)", p=P, j=nr)
    rgbv = rgb_out.rearrange("(n p j) c -> n p (j c)", p=P, j=nr)
    depv = dep_out.rearrange("(n p j) c -> n p (j c)", p=P, j=nr)

    with tile.TileContext(nc) as tc, ExitStack() as ctx:
        def pool(name, bufs):
            return ctx.enter_context(tc.tile_pool(name=name, bufs=bufs))

        p_c = pool("colors", 2)
        p_dl = pool("dl", 2)
        p_d = pool("d", 4)
        p_dlsum = pool("dlsum", 2)
        p_e1 = pool("e1", 2)
        p_sp = pool("sp", 2)
        p_dneg = pool("dneg", 2)
        p_dd = pool("dd", 2)
        p_em = pool("em", 2)
        p_al = pool("al", 2)
        p_ae = pool("ae", 2)
        p_d0 = pool("d0", 3)
        p_tt = pool("tt", 3)
        p_w = pool("w", 3)
        p_dms = pool("dms", 2)
        p_v = pool("v", 2)
        p_scr = pool("scr", 2)
        p_sm = pool("sm", 3)

        st1 = {}
        st2 = {}

        def s1(t):
            """Loads + everything that depends only on the loads."""
            dlt = p_dl.tile([P, FS], f32)
            nc.sync.dma_start(dlt[:], dlv[t])
            dt = p_d.tile([P, FS], f32)
            nc.sync.dma_start(dt[:], dv[t])

            # flat shifted adds (garbage in ray-boundary lanes)
            dlsum = p_dlsum.tile([P, FS], f32)
            nc.gpsimd.tensor_tensor(
                dlsum[:, 0:FSm], dlt[:, 0:FSm], dlt[:, 1:FS], ALU.add
            )
            dneg = p_dneg.tile([P, FS], f32)
            nc.gpsimd.tensor_tensor(
                dneg[:, 0:FSm], dt[:, 0:FSm], dt[:, 1:FS], ALU.subtract
            )
            dms = p_dms.tile([P, FS], f32)
            nc.gpsimd.tensor_tensor(
                dms[:, 0:FSm], dt[:, 0:FSm], dt[:, 1:FS], ALU.add
            )

            # densities_mid = softplus(0.5*(dl[s]+dl[s+1]) - 1)
            #               = Ln(1 + Exp(0.5*dl_sum - 1))
            e1 = p_e1.tile([P, FS], f32)
            nc.scalar.activation(
                e1[:, 0:FSm], dlsum[:, 0:FSm], AF.Exp, scale=0.5, bias=-1.0
            )
            sp = p_sp.tile([P, FS], f32)
            nc.scalar.activation(sp[:, 0:FSm], e1[:, 0:FSm], AF.Ln, bias=1.0)

            st1[t] = (dt, sp, dneg, dms)

        def s2(t):
            """Transcendental chain, transmittance scan, weights."""
            dt, sp, dneg, dms = st1.pop(t)
            ct = p_c.tile([P, FC], f32)
            nc.sync.dma_start(ct[:], cv[t])

            dd = p_dd.tile([P, FS], f32)
            nc.vector.tensor_mul(dd[:, 0:FSm], sp[:, 0:FSm], dneg[:, 0:FSm])
            em = p_em.tile([P, FS], f32)
            nc.scalar.activation(em[:, 0:FSm], dd[:, 0:FSm], AF.Exp)
            em3 = em[:].rearrange("p (j s) -> p j s", j=nr)

            # alpha = 1 - em (output), aeps = alpha + EPS
            al = p_al.tile([P, FS], f32)
            nc.scalar.activation(
                al[:, 0:FSm], em[:, 0:FSm], AF.Identity, scale=-1.0, bias=1.0
            )
            al3 = al[:].rearrange("p (j s) -> p j s", j=nr)
            nc.scalar.dma_start(av[t], al3[:, :, 0:SM1])
            ae = p_ae.tile([P, FS], f32)
            nc.scalar.activation(
                ae[:, 0:FSm], al[:, 0:FSm], AF.Identity, bias=EPS
            )

            # segmented transmittance scan input: d0[s] = em[s-1] + EPS
            # (shifted by one, flat), with 0 at every ray start (memset on
            # the vector engine so GPSIMD's queue stays stall-free).
            d0 = p_d0.tile([P, FS], f32)
            d03 = d0[:].rearrange("p (j s) -> p j s", j=nr)
            nc.scalar.mul(d03[:, :, 0:1], reset_ap.rearrange("p (j s) -> p j s", j=nr)[:, :, 0:1], 0.0)
            nc.scalar.activation(
                d03[:, :, 1:S], em3[:, :, 0:SM1], AF.Identity, bias=EPS
            )

            # T[j, s] = prod_{k<s} (1 - alpha[j,k] + EPS);  T[j, 0] = 1.
            tt = p_tt.tile([P, FS], f32)
            nc.vector.tensor_tensor_scan(
                out=tt[:],
                data0=d0[:],
                data1=reset_ap,
                initial=1.0,
                op0=ALU.mult,
                op1=ALU.add,
            )
            tt3 = tt[:].rearrange("p (j s) -> p j s", j=nr)

            # weights (flat mul; garbage boundary lane zeroed so the later
            # shifted add for v reads defined memory)
            w = p_w.tile([P, FS], f32)
            nc.vector.tensor_mul(w[:, 0:FSm], ae[:, 0:FSm], tt[:, 0:FSm])
            w3 = w[:].rearrange("p (j s) -> p j s", j=nr)
            nc.scalar.dma_start(wv[t], w3[:, :, 0:SM1])

            # weight_total by telescoping: wt = 1 - T[127]; denominator of
            # composite_depth folded as 2*(EPS + wt) = 2*(1 - T[127]) + 2EPS
            wtm = p_sm.tile([P, nr], f32, tag="wtm")
            nc.scalar.activation(
                wtm[:], tt3[:, :, SM1:S], AF.Identity, scale=-1.0, bias=1.0
            )
            wt2 = p_sm.tile([P, nr], f32, tag="wt2")
            nc.scalar.activation(
                wt2[:], wtm[:], AF.Identity, scale=2.0, bias=2.0 * EPS
            )

            # depths_mid = 0.5*(d[s]+d[s+1]) (output), scaled in place
            nc.scalar.mul(dms[:, 0:FSm], dms[:, 0:FSm], 0.5)
            dms3 = dms[:].rearrange("p (j s) -> p j s", j=nr)
            nc.scalar.dma_start(dmv[t], dms3[:, :, 0:SM1])

            st2[t] = (ct, dt, w, wt2)

        def s3(t):
            """v, products, reductions, composites."""
            ct, dt, w, wt2 = st2.pop(t)
            w3 = w[:].rearrange("p (j s) -> p j s", j=nr)
            cflat = ct[:].rearrange("p (f c) -> p f c", c=3)

            # v[s] = w[s-1] + w[s] with one-sided boundaries: the composite
            # sums become  rgb = sum_s v[s]*c[s] - 1  and
            # depth_num = 0.5 * sum_s v[s]*d[s].  Flat add + boundary fixups.
            v = p_v.tile([P, FS], f32)
            nc.gpsimd.tensor_tensor(v[:, 1:FSm], w[:, 0 : FSm - 1], w[:, 1:FSm], ALU.add)
            v3 = v[:].rearrange("p (j s) -> p j s", j=nr)
            nc.scalar.copy(v3[:, :, 0:1], w3[:, :, 0:1])
            nc.scalar.copy(v3[:, :, SM1:S], w3[:, :, SM1 - 1 : SM1])

            # products + per-ray reductions
            pr_vd = p_scr.tile([P, FS], f32, tag="pr_vd")
            pr_dve = p_scr.tile([P, FS], f32, tag="pr_dve")
            prv3 = pr_vd[:].rearrange("p (j s) -> p j s", j=nr)
            prd3 = pr_dve[:].rearrange("p (j s) -> p j s", j=nr)

            dsum = p_sm.tile([P, nr], f32, tag="dsum")
            nc.gpsimd.tensor_tensor(pr_vd[:], v[:], dt[:], ALU.mult)
            nc.vector.tensor_reduce(
                out=dsum[:], in_=prv3, axis=mybir.AxisListType.X, op=ALU.add
            )

            rgbacc = p_sm.tile([P, nr * 3], f32, tag="rgbacc")
            racc3 = rgbacc[:].rearrange("p (j c) -> p j c", j=nr)
            for ch in (0, 1, 2):
                nc.vector.tensor_tensor(pr_dve[:], v[:], cflat[:, :, ch], ALU.mult)
                nc.vector.tensor_reduce(
                    out=racc3[:, :, ch : ch + 1],
                    in_=prd3,
                    axis=mybir.AxisListType.X,
                    op=ALU.add,
                )
            rgbt = p_sm.tile([P, nr * 3], f32, tag="rgb")
            nc.scalar.activation(rgbt[:], rgbacc[:], AF.Identity, bias=-1.0)
            nc.scalar.dma_start(rgbv[t], rgbt[:])

            # composite_depth = (0.5*dsum) / (EPS + wt), clipped with exact
            # min/max as 1-port tensor_tensor ops (no GPSIMD port contention)
            rec = p_sm.tile([P, nr], f32, tag="rec")
            nc.vector.reciprocal(rec[:], wt2[:])
            dep = p_sm.tile([P, nr], f32, tag="dep")
            nc.vector.tensor_mul(dep[:], dsum[:], rec[:])
            dep2 = p_sm.tile([P, nr], f32, tag="dep2")
            nc.vector.tensor_tensor(dep2[:], dep[:], clip_lo, ALU.max)
            nc.vector.tensor_tensor(dep2[:], dep2[:], clip_hi, ALU.min)
            nc.scalar.dma_start(depv[t], dep2[:])

        s1(0)
        if n_tiles > 1:
            s1(1)
        for t in range(n_tiles):
            if t + 2 < n_tiles:
                s1(t + 2)
            s2(t)
            if t >= 1:
                s3(t - 1)
        s3(n_tiles - 1)

    nc.compile()
    return nc


def _get_program():
    key = (PER_CORE, NR)
    if key not in _PROGRAMS:
        _PROGRAMS[key] = build_program(*key)
    return _PROGRAMS[key]


def kernel(colors, density_logits, depths):
    from concourse.bass_utils import run_bass_kernel_spmd

    colors = np.ascontiguousarray(np.asarray(colors, dtype=np.float32))
    dl = np.ascontiguousarray(np.asarray(density_logits, dtype=np.float32))
    d = np.ascontiguousarray(np.asarray(depths, dtype=np.float32))

    cf = colors.reshape(G, S, 3)
    dlf = dl.reshape(G, S)
    df = d.reshape(G, S)

    nc = _get_program()
    in_maps = [
        {
            "colors": cf[c * PER_CORE : (c + 1) * PER_CORE],
            "density_logits": dlf[c * PER_CORE : (c + 1) * PER_CORE],
            "depths": df[c * PER_CORE : (c + 1) * PER_CORE],
        }
        for c in range(N_CORES)
    ]
    res = run_bass_kernel_spmd(nc, in_maps, list(range(N_CORES))).results

    w = np.concatenate([r["weights"] for r in res]).reshape(B, R, SM1, 1)
    al = np.concatenate([r["alpha"] for r in res]).reshape(B, R, SM1, 1)
    dm = np.concatenate([r["depths_mid"] for r in res]).reshape(B, R, SM1, 1)
    rgb = np.concatenate([r["rgb"] for r in res]).reshape(B, R, 3)
    dep = np.concatenate([r["depth"] for r in res]).reshape(B, R, 1)

    # weights_all is the same array as weights in the reference.
    return (rgb, dep, w, w, al, dm)


# revision 12
# speedup vs baseline: 1.2646x; 1.2646x over previous
"""Trainium2 Bass/Tile kernel for the MipRayMarcher nn.Module.

Full inputs:
    colors         [4, 16384, 128, 3] f32
    density_logits [4, 16384, 128, 1] f32
    depths         [4, 16384, 128, 1] f32  (sorted along samples)

Returns the reference tuple:
    (composite_rgb [B,R,3], composite_depth [B,R,1], weights [B,R,127,1],
     weights_all (= weights), alpha [B,R,127,1], depths_mid [B,R,127,1])

Strategy: flatten (B, R) -> 65536 rays, shard 8192 contiguous rays per
NeuronCore (data parallel).  Per core, rays map ray-major into SBUF tiles of
[128 partitions x NR=8 ray slots x 128 samples], and every elementwise op
runs on a single flat free dimension (full engine rate).  Positions at ray
boundaries (sample 127 of each slot) carry garbage that is either never
read, excluded by strided output DMAs, or patched by small fixup copies.

The per-ray transmittance cumprod runs as ONE segmented DVE
tensor_tensor_scan per tile: state = d0[t]*state + d1[t], where d1 is a
constant reset pattern (1 at each ray's sample 0) and d0 carries
(1-alpha+EPS) shifted by one sample with 0 at ray starts.  weight_total
comes from the scan's per-ray end state by telescoping
(wt = 1 - T[127] + O(1e-8)), eliminating a full reduction.

Engine placement (measured costs): GPSIMD takes the five flat shifted
adds/products; the vector engine takes the scan, the strided color
products, all reductions, and small exact min/max clips (as 1-port
tensor_tensor ops against broadcast constants, which never contend with
GPSIMD for the shared SBUF port); ScalarE takes all transcendentals off a
single activation table set.  Work is emitted as a 3-stage software
pipeline (s1 two tiles ahead, s2 one tile ahead, s3 current) so the
in-order engine queues always hold ready work instead of head-of-line
blocking on cross-engine dependencies.
"""

import numpy as np

B, R, S = 4, 16384, 128
SM1 = S - 1
N_CORES = 8
G = B * R                      # 65536 rays
PER_CORE = G // N_CORES        # 8192 rays per core
P = 128                        # SBUF partitions
NR = 8                         # ray slots per partition per tile
TILE_RAYS = P * NR             # 1024 rays per tile
EPS = 1e-10
MIN_DEPTH, MAX_DEPTH = 0.1, 100.0

_PROGRAMS = {}


def build_program(per_core=PER_CORE, nr=NR):
    from contextlib import ExitStack

    import concourse.bacc as bacc
    import concourse.mybir as mybir
    import concourse.tile as tile
    from concourse.hw_specs import get_activation_tables

    AF = mybir.ActivationFunctionType
    ALU = mybir.AluOpType
    f32 = mybir.dt.float32

    # Keep Exp and Ln resolvable only from the combined table set, so the
    # whole kernel needs exactly one ACT table load (the selection pass is
    # greedy-first otherwise and would thrash between sets).
    tabs = get_activation_tables("gen3")
    for name, funcs in tabs.items():
        if name != "natural_log_exp_and_others":
            funcs.discard(AF.Exp)
            funcs.discard(AF.Ln)

    n_tiles = per_core // (P * nr)
    assert n_tiles * P * nr == per_core
    FS = nr * S          # flat free elems per dl/depth tile (1024)
    FSm = FS - 1         # flat shifted-op length (1023)
    FC = nr * S * 3      # colors tile free elems

    nc = bacc.Bacc("TRN2", target_bir_lowering=False, debug=False)

    def register_const(v):
        t = nc.alloc_sbuf_tensor(f"constf32-{v}", [P, 1], f32)
        nc.gpsimd.memset(t.ap(), v)
        nc.const_aps.aps[(f32, v)] = t.ap()

    for v in (-1.0, EPS, 2.0 * EPS, MIN_DEPTH, MAX_DEPTH):
        register_const(v)

    # Constant scan-reset pattern: 1.0 at each ray slot's sample 0, else 0.
    reset_t = nc.alloc_sbuf_tensor("scan_reset", [P, FS], f32)
    reset_ap = reset_t.ap()
    nc.gpsimd.memset(reset_ap, 0.0)
    nc.all_engine_barrier()
    nc.gpsimd.memset(reset_ap.rearrange("p (j s) -> p j s", j=nr)[:, :, 0:1], 1.0)
    nc.all_engine_barrier()

    clip_lo = nc.const_aps.tensor(MIN_DEPTH, (P, nr))
    clip_hi = nc.const_aps.tensor(MAX_DEPTH, (P, nr))

    colors_in = nc.dram_tensor("colors", [per_core, S, 3], f32, kind="ExternalInput").ap()
    dl_in = nc.dram_tensor("density_logits", [per_core, S], f32, kind="ExternalInput").ap()
    d_in = nc.dram_tensor("depths", [per_core, S], f32, kind="ExternalInput").ap()
    w_out = nc.dram_tensor("weights", [per_core, SM1], f32, kind="ExternalOutput").ap()
    a_out = nc.dram_tensor("alpha", [per_core, SM1], f32, kind="ExternalOutput").ap()
    dm_out = nc.dram_tensor("depths_mid", [per_core, SM1], f32, kind="ExternalOutput").ap()
    rgb_out = nc.dram_tensor("rgb", [per_core, 3], f32, kind="ExternalOutput").ap()
    dep_out = nc.dram_tensor("depth", [per_core, 1], f32, kind="ExternalOutput").ap()

    cv = colors_in.rearrange("(n p j) s c -> n p (j s c)", p=P, j=nr)
    dlv = dl_in.rearrange("(n p j) s -> n p (j s)", p=P, j=nr)
    dv = d_in.rearrange("(n p j) s -> n p (j s)", p=P, j=nr)
    wv = w_out.rearrange("(n p j) s -> n p (j s)", p=P, j=nr)
    av = a_out.rearrange("(n p j) s -> n p (j s)", p=P, j=nr)
    dmv = dm_out.rearrange("(n p j) s -> n p (j s)", p=P, j=nr)
    rgbv = rgb_out.rearrange("(n p j) c -> n p (j c)", p=P, j=nr)
    depv = dep_out.rearrange("(n p j) c -> n p (j c)", p=P, j=nr)

    with tile.TileContext(nc) as tc, ExitStack() as ctx:
        def pool(name, bufs):
            return ctx.enter_context(tc.tile_pool(name=name, bufs=bufs))

        p_c = pool("colors", 3)
        p_dl = pool("dl", 2)
        p_d = pool("d", 4)
        p_dlsum = pool("dlsum", 2)
        p_e1 = pool("e1", 2)
        p_sp = pool("sp", 2)
        p_dneg = pool("dneg", 2)
        p_dd = pool("dd", 2)
        p_em = pool("em", 2)
        p_al = pool("al", 2)
        p_ae = pool("ae", 2)
        p_d0 = pool("d0", 2)
        p_tt = pool("tt", 2)
        p_w = pool("w", 3)
        p_dms = pool("dms", 2)
        p_v = pool("v", 2)
        p_scr = pool("scr", 2)
        p_sm = pool("sm", 3)

        st1 = {}
        st2 = {}

        def s1(t):
            """Loads + everything that depends only on the loads."""
            ct = p_c.tile([P, FC], f32)
            nc.sync.dma_start(ct[:], cv[t])
            dlt = p_dl.tile([P, FS], f32)
            nc.sync.dma_start(dlt[:], dlv[t])
            dt = p_d.tile([P, FS], f32)
            nc.sync.dma_start(dt[:], dv[t])

            # flat shifted adds (garbage in ray-boundary lanes)
            dlsum = p_dlsum.tile([P, FS], f32)
            nc.gpsimd.tensor_tensor(
                dlsum[:, 0:FSm], dlt[:, 0:FSm], dlt[:, 1:FS], ALU.add
            )
            dneg = p_dneg.tile([P, FS], f32)
            nc.gpsimd.tensor_tensor(
                dneg[:, 0:FSm], dt[:, 0:FSm], dt[:, 1:FS], ALU.subtract
            )
            dms = p_dms.tile([P, FS], f32)
            nc.gpsimd.tensor_tensor(
                dms[:, 0:FSm], dt[:, 0:FSm], dt[:, 1:FS], ALU.add
            )

            # densities_mid = softplus(0.5*(dl[s]+dl[s+1]) - 1)
            #               = Ln(1 + Exp(0.5*dl_sum - 1))
            e1 = p_e1.tile([P, FS], f32)
            nc.scalar.activation(
                e1[:, 0:FSm], dlsum[:, 0:FSm], AF.Exp, scale=0.5, bias=-1.0
            )
            sp = p_sp.tile([P, FS], f32)
            nc.scalar.activation(sp[:, 0:FSm], e1[:, 0:FSm], AF.Ln, bias=1.0)

            st1[t] = (ct, dt, sp, dneg, dms)

        def s2(t):
            """Transcendental chain, transmittance scan, weights."""
            ct, dt, sp, dneg, dms = st1.pop(t)

            dd = p_dd.tile([P, FS], f32)
            nc.vector.tensor_mul(dd[:, 0:FSm], sp[:, 0:FSm], dneg[:, 0:FSm])
            em = p_em.tile([P, FS], f32)
            nc.scalar.activation(em[:, 0:FSm], dd[:, 0:FSm], AF.Exp)
            em3 = em[:].rearrange("p (j s) -> p j s", j=nr)

            # alpha = 1 - em (output), aeps = alpha + EPS
            al = p_al.tile([P, FS], f32)
            nc.scalar.activation(
                al[:, 0:FSm], em[:, 0:FSm], AF.Identity, scale=-1.0, bias=1.0
            )
            al3 = al[:].rearrange("p (j s) -> p j s", j=nr)
            nc.scalar.dma_start(av[t], al3[:, :, 0:SM1])
            ae = p_ae.tile([P, FS], f32)
            nc.scalar.activation(
                ae[:, 0:FSm], al[:, 0:FSm], AF.Identity, bias=EPS
            )

            # segmented transmittance scan input: d0[s] = em[s-1] + EPS
            # (shifted by one, flat), with 0 at every ray start (memset on
            # the vector engine so GPSIMD's queue stays stall-free).
            d0 = p_d0.tile([P, FS], f32)
            nc.scalar.activation(
                d0[:, 1:FS], em[:, 0:FSm], AF.Identity, bias=EPS
            )
            d03 = d0[:].rearrange("p (j s) -> p j s", j=nr)
            nc.vector.memset(d03[:, :, 0:1], 0.0)

            # T[j, s] = prod_{k<s} (1 - alpha[j,k] + EPS);  T[j, 0] = 1.
            tt = p_tt.tile([P, FS], f32)
            nc.vector.tensor_tensor_scan(
                out=tt[:],
                data0=d0[:],
                data1=reset_ap,
                initial=1.0,
                op0=ALU.mult,
                op1=ALU.add,
            )
            tt3 = tt[:].rearrange("p (j s) -> p j s", j=nr)

            # weights (flat mul; garbage boundary lane zeroed so the later
            # shifted add for v reads defined memory)
            w = p_w.tile([P, FS], f32)
            nc.vector.memset(w[:, FSm:FS], 0.0)
            nc.vector.tensor_mul(w[:, 0:FSm], ae[:, 0:FSm], tt[:, 0:FSm])
            w3 = w[:].rearrange("p (j s) -> p j s", j=nr)
            nc.scalar.dma_start(wv[t], w3[:, :, 0:SM1])

            # weight_total by telescoping: wt = 1 - T[127]; denominator of
            # composite_depth folded as 2*(EPS + wt) = 2*(1 - T[127]) + 2EPS
            wtm = p_sm.tile([P, nr], f32, tag="wtm")
            nc.scalar.activation(
                wtm[:], tt3[:, :, SM1:S], AF.Identity, scale=-1.0, bias=1.0
            )
            wt2 = p_sm.tile([P, nr], f32, tag="wt2")
            nc.scalar.activation(
                wt2[:], wtm[:], AF.Identity, scale=2.0, bias=2.0 * EPS
            )

            # depths_mid = 0.5*(d[s]+d[s+1]) (output), scaled in place
            nc.scalar.mul(dms[:, 0:FSm], dms[:, 0:FSm], 0.5)
            dms3 = dms[:].rearrange("p (j s) -> p j s", j=nr)
            nc.scalar.dma_start(dmv[t], dms3[:, :, 0:SM1])

            st2[t] = (ct, dt, w, wt2)

        def s3(t):
            """v, products, reductions, composites."""
            ct, dt, w, wt2 = st2.pop(t)
            w3 = w[:].rearrange("p (j s) -> p j s", j=nr)
            cflat = ct[:].rearrange("p (f c) -> p f c", c=3)

            # v[s] = w[s-1] + w[s] with one-sided boundaries: the composite
            # sums become  rgb = sum_s v[s]*c[s] - 1  and
            # depth_num = 0.5 * sum_s v[s]*d[s].  Flat add + boundary fixups.
            v = p_v.tile([P, FS], f32)
            nc.gpsimd.tensor_tensor(v[:, 1:FS], w[:, 0:FSm], w[:, 1:FS], ALU.add)
            v3 = v[:].rearrange("p (j s) -> p j s", j=nr)
            nc.scalar.copy(v3[:, :, 0:1], w3[:, :, 0:1])
            nc.scalar.copy(v3[:, :, SM1:S], w3[:, :, SM1 - 1 : SM1])

            # products + per-ray reductions
            pr_vd = p_scr.tile([P, FS], f32, tag="pr_vd")
            pr_dve = p_scr.tile([P, FS], f32, tag="pr_dve")
            prv3 = pr_vd[:].rearrange("p (j s) -> p j s", j=nr)
            prd3 = pr_dve[:].rearrange("p (j s) -> p j s", j=nr)

            dsum = p_sm.tile([P, nr], f32, tag="dsum")
            nc.gpsimd.tensor_tensor(pr_vd[:], v[:], dt[:], ALU.mult)
            nc.vector.tensor_reduce(
                out=dsum[:], in_=prv3, axis=mybir.AxisListType.X, op=ALU.add
            )

            rgbacc = p_sm.tile([P, nr * 3], f32, tag="rgbacc")
            racc3 = rgbacc[:].rearrange("p (j c) -> p j c", j=nr)
            for ch in (0, 1, 2):
                nc.vector.tensor_tensor(pr_dve[:], v[:], cflat[:, :, ch], ALU.mult)
                nc.vector.tensor_reduce(
                    out=racc3[:, :, ch : ch + 1],
                    in_=prd3,
                    axis=mybir.AxisListType.X,
                    op=ALU.add,
                )
            rgbt = p_sm.tile([P, nr * 3], f32, tag="rgb")
            nc.scalar.activation(rgbt[:], rgbacc[:], AF.Identity, bias=-1.0)
            nc.scalar.dma_start(rgbv[t], rgbt[:])

            # composite_depth = (0.5*dsum) / (EPS + wt), clipped with exact
            # min/max as 1-port tensor_tensor ops (no GPSIMD port contention)
            rec = p_sm.tile([P, nr], f32, tag="rec")
            nc.vector.reciprocal(rec[:], wt2[:])
            dep = p_sm.tile([P, nr], f32, tag="dep")
            nc.vector.tensor_mul(dep[:], dsum[:], rec[:])
            dep2 = p_sm.tile([P, nr], f32, tag="dep2")
            nc.vector.tensor_tensor(dep2[:], dep[:], clip_lo, ALU.max)
            nc.vector.tensor_tensor(dep2[:], dep2[:], clip_hi, ALU.min)
            nc.scalar.dma_start(depv[t], dep2[:])

        s1(0)
        if n_tiles > 1:
            s1(1)
        for t in range(n_tiles):
            if t + 2 < n_tiles:
                s1(t + 2)
            s2(t)
            if t >= 1:
                s3(t - 1)
        s3(n_tiles - 1)

    nc.compile()
    return nc


def _get_program():
    key = (PER_CORE, NR)
    if key not in _PROGRAMS:
        _PROGRAMS[key] = build_program(*key)
    return _PROGRAMS[key]


def kernel(colors, density_logits, depths):
    from concourse.bass_utils import run_bass_kernel_spmd

    colors = np.ascontiguousarray(np.asarray(colors, dtype=np.float32))
    dl = np.ascontiguousarray(np.asarray(density_logits, dtype=np.float32))
    d = np.ascontiguousarray(np.asarray(depths, dtype=np.float32))

    cf = colors.reshape(G, S, 3)
    dlf = dl.reshape(G, S)
    df = d.reshape(G, S)

    nc = _get_program()
    in_maps = [
        {
            "colors": cf[c * PER_CORE : (c + 1) * PER_CORE],
            "density_logits": dlf[c * PER_CORE : (c + 1) * PER_CORE],
            "depths": df[c * PER_CORE : (c + 1) * PER_CORE],
        }
        for c in range(N_CORES)
    ]
    res = run_bass_kernel_spmd(nc, in_maps, list(range(N_CORES))).results

    w = np.concatenate([r["weights"] for r in res]).reshape(B, R, SM1, 1)
    al = np.concatenate([r["alpha"] for r in res]).reshape(B, R, SM1, 1)
    dm = np.concatenate([r["depths_mid"] for r in res]).reshape(B, R, SM1, 1)
    rgb = np.concatenate([r["rgb"] for r in res]).reshape(B, R, 3)
    dep = np.concatenate([r["depth"] for r in res]).reshape(B, R, 1)

    # weights_all is the same array as weights in the reference.
    return (rgb, dep, w, w, al, dm)


# revision 14
# speedup vs baseline: 1.2775x; 1.0102x over previous
"""Trainium2 Bass/Tile kernel for the MipRayMarcher nn.Module.

Full inputs:
    colors         [4, 16384, 128, 3] f32
    density_logits [4, 16384, 128, 1] f32
    depths         [4, 16384, 128, 1] f32  (sorted along samples)

Returns the reference tuple:
    (composite_rgb [B,R,3], composite_depth [B,R,1], weights [B,R,127,1],
     weights_all (= weights), alpha [B,R,127,1], depths_mid [B,R,127,1])

Strategy: flatten (B, R) -> 65536 rays, shard 8192 contiguous rays per
NeuronCore (data parallel).  Per core, rays map ray-major into SBUF tiles of
[128 partitions x NR=8 ray slots x 128 samples], and every elementwise op
runs on a single flat free dimension (full engine rate).  Positions at ray
boundaries (sample 127 of each slot) carry garbage that is either never
read, excluded by strided output DMAs, or patched by small fixup copies.

The per-ray transmittance cumprod runs as ONE segmented DVE
tensor_tensor_scan per tile: state = d0[t]*state + d1[t], where d1 is a
constant reset pattern (1 at each ray's sample 0) and d0 carries
(1-alpha+EPS) shifted by one sample with 0 at ray starts.  weight_total
comes from the scan's per-ray end state by telescoping
(wt = 1 - T[127] + O(1e-8)), eliminating a full reduction.

Engine placement (measured costs): GPSIMD takes the five flat shifted
adds/products; the vector engine takes the scan, the strided color
products, all reductions, and small exact min/max clips (as 1-port
tensor_tensor ops against broadcast constants, which never contend with
GPSIMD for the shared SBUF port); ScalarE takes all transcendentals off a
single activation table set.  Work is emitted as a 3-stage software
pipeline (s1 two tiles ahead, s2 one tile ahead, s3 current) so the
in-order engine queues always hold ready work instead of head-of-line
blocking on cross-engine dependencies.
"""

import numpy as np

B, R, S = 4, 16384, 128
SM1 = S - 1
N_CORES = 8
G = B * R                      # 65536 rays
PER_CORE = G // N_CORES        # 8192 rays per core
P = 128                        # SBUF partitions
NR = 8                         # ray slots per partition per tile
TILE_RAYS = P * NR             # 1024 rays per tile
EPS = 1e-10
MIN_DEPTH, MAX_DEPTH = 0.1, 100.0

_PROGRAMS = {}


def build_program(per_core=PER_CORE, nr=NR):
    from contextlib import ExitStack

    import concourse.bacc as bacc
    import concourse.mybir as mybir
    import concourse.tile as tile
    from concourse.hw_specs import get_activation_tables

    AF = mybir.ActivationFunctionType
    ALU = mybir.AluOpType
    f32 = mybir.dt.float32

    # Keep Exp and Ln resolvable only from the combined table set, so the
    # whole kernel needs exactly one ACT table load (the selection pass is
    # greedy-first otherwise and would thrash between sets).
    tabs = get_activation_tables("gen3")
    for name, funcs in tabs.items():
        if name != "natural_log_exp_and_others":
            funcs.discard(AF.Exp)
            funcs.discard(AF.Ln)

    n_tiles = per_core // (P * nr)
    assert n_tiles * P * nr == per_core
    FS = nr * S          # flat free elems per dl/depth tile (1024)
    FSm = FS - 1         # flat shifted-op length (1023)
    FC = nr * S * 3      # colors tile free elems

    nc = bacc.Bacc("TRN2", target_bir_lowering=False, debug=False)

    def register_const(v):
        t = nc.alloc_sbuf_tensor(f"constf32-{v}", [P, 1], f32)
        nc.gpsimd.memset(t.ap(), v)
        nc.const_aps.aps[(f32, v)] = t.ap()

    for v in (-1.0, 2.0, EPS, 2.0 * EPS, MIN_DEPTH, MAX_DEPTH):
        register_const(v)

    # Constant scan-reset pattern: 1.0 at each ray slot's sample 0, else 0.
    reset_t = nc.alloc_sbuf_tensor("scan_reset", [P, FS], f32)
    reset_ap = reset_t.ap()
    nc.gpsimd.memset(reset_ap, 0.0)
    nc.all_engine_barrier()
    nc.gpsimd.memset(reset_ap.rearrange("p (j s) -> p j s", j=nr)[:, :, 0:1], 1.0)
    nc.all_engine_barrier()

    clip_lo = nc.const_aps.tensor(MIN_DEPTH, (P, nr))
    clip_hi = nc.const_aps.tensor(MAX_DEPTH, (P, nr))

    colors_in = nc.dram_tensor("colors", [per_core, S, 3], f32, kind="ExternalInput").ap()
    dl_in = nc.dram_tensor("density_logits", [per_core, S], f32, kind="ExternalInput").ap()
    d_in = nc.dram_tensor("depths", [per_core, S], f32, kind="ExternalInput").ap()
    w_out = nc.dram_tensor("weights", [per_core, SM1], f32, kind="ExternalOutput").ap()
    a_out = nc.dram_tensor("alpha", [per_core, SM1], f32, kind="ExternalOutput").ap()
    dm_out = nc.dram_tensor("depths_mid", [per_core, SM1], f32, kind="ExternalOutput").ap()
    rgb_out = nc.dram_tensor("rgb", [per_core, 3], f32, kind="ExternalOutput").ap()
    dep_out = nc.dram_tensor("depth", [per_core, 1], f32, kind="ExternalOutput").ap()

    cv = colors_in.rearrange("(n p j) s c -> n p (j s c)", p=P, j=nr)
    dlv = dl_in.rearrange("(n p j) s -> n p (j s)", p=P, j=nr)
    dv = d_in.rearrange("(n p j) s -> n p (j s)", p=P, j=nr)
    wv = w_out.rearrange("(n p j) s -> n p (j s)", p=P, j=nr)
    av = a_out.rearrange("(n p j) s -> n p (j s)", p=P, j=nr)
    dmv = dm_out.rearrange("(n p j) s -> n p (j s)", p=P, j=nr)
    rgbv = rgb_out.rearrange("(n p j) c -> n p (j c)", p=P, j=nr)
    depv = dep_out.rearrange("(n p j) c -> n p (j c)", p=P, j=nr)

    with tile.TileContext(nc) as tc, ExitStack() as ctx:
        def pool(name, bufs):
            return ctx.enter_context(tc.tile_pool(name=name, bufs=bufs))

        p_c = pool("colors", 3)
        p_dl = pool("dl", 2)
        p_d = pool("d", 4)
        p_dlsum = pool("dlsum", 2)
        p_sp = pool("sp", 2)
        p_dneg = pool("dneg", 2)
        p_dd = pool("dd", 2)
        p_em = pool("em", 2)
        p_al = pool("al", 2)
        p_d0 = pool("d0", 3)
        p_tt = pool("tt", 3)
        p_w = pool("w", 3)
        p_dms = pool("dms", 2)
        p_v = pool("v", 2)
        p_scr = pool("scr", 2)
        p_sm = pool("sm", 3)

        st1 = {}
        st2 = {}

        def s1(t):
            """Loads + everything that depends only on the loads."""
            ct = p_c.tile([P, FC], f32)
            nc.sync.dma_start(ct[:], cv[t])
            dlt = p_dl.tile([P, FS], f32)
            nc.sync.dma_start(dlt[:], dlv[t])
            dt = p_d.tile([P, FS], f32)
            nc.sync.dma_start(dt[:], dv[t])

            # flat shifted adds (garbage in ray-boundary lanes)
            dlsum = p_dlsum.tile([P, FS], f32)
            nc.gpsimd.tensor_tensor(
                dlsum[:, 0:FSm], dlt[:, 0:FSm], dlt[:, 1:FS], ALU.add
            )
            dneg = p_dneg.tile([P, FS], f32)
            nc.gpsimd.tensor_tensor(
                dneg[:, 0:FSm], dt[:, 0:FSm], dt[:, 1:FS], ALU.subtract
            )
            dms = p_dms.tile([P, FS], f32)
            nc.gpsimd.tensor_tensor(
                dms[:, 0:FSm], dt[:, 0:FSm], dt[:, 1:FS], ALU.add
            )

            # densities_mid = softplus(0.5*(dl[s]+dl[s+1]) - 1)
            #               = Ln(1 + Exp(0.5*dl_sum - 1))
            sp = p_sp.tile([P, FS], f32)
            nc.scalar.activation(
                sp[:, 0:FSm], dlsum[:, 0:FSm], AF.Exp, scale=0.5, bias=-1.0
            )
            nc.scalar.activation(sp[:, 0:FSm], sp[:, 0:FSm], AF.Ln, bias=1.0)

            st1[t] = (ct, dt, sp, dneg, dms)

        def s2(t):
            """Transcendental chain, transmittance scan, weights."""
            ct, dt, sp, dneg, dms = st1.pop(t)

            dd = p_dd.tile([P, FS], f32)
            nc.vector.tensor_mul(dd[:, 0:FSm], sp[:, 0:FSm], dneg[:, 0:FSm])
            em = p_em.tile([P, FS], f32)
            nc.scalar.activation(em[:, 0:FSm], dd[:, 0:FSm], AF.Exp)
            em3 = em[:].rearrange("p (j s) -> p j s", j=nr)

            # alpha = 1 - em (output), aeps = alpha + EPS
            al = p_al.tile([P, FS], f32)
            nc.scalar.activation(
                al[:, 0:FSm], em[:, 0:FSm], AF.Identity, scale=-1.0, bias=1.0
            )
            al3 = al[:].rearrange("p (j s) -> p j s", j=nr)
            nc.scalar.dma_start(av[t], al3[:, :, 0:SM1])
            # segmented transmittance scan input: d0[s] = em[s-1] + EPS
            # (shifted by one, flat), with 0 at every ray start (memset on
            # the vector engine so GPSIMD's queue stays stall-free).
            d0 = p_d0.tile([P, FS], f32)
            nc.scalar.activation(
                d0[:, 1:FS], em[:, 0:FSm], AF.Identity, bias=EPS
            )
            d03 = d0[:].rearrange("p (j s) -> p j s", j=nr)
            nc.vector.memset(d03[:, :, 0:1], 0.0)

            # T[j, s] = prod_{k<s} (1 - alpha[j,k] + EPS);  T[j, 0] = 1.
            tt = p_tt.tile([P, FS], f32)
            nc.vector.tensor_tensor_scan(
                out=tt[:],
                data0=d0[:],
                data1=reset_ap,
                initial=1.0,
                op0=ALU.mult,
                op1=ALU.add,
            )
            tt3 = tt[:].rearrange("p (j s) -> p j s", j=nr)

            # weights (flat mul; garbage boundary lane zeroed so the later
            # shifted add for v reads defined memory)
            w = p_w.tile([P, FS], f32)
            nc.vector.memset(w[:, FSm:FS], 0.0)
            nc.vector.tensor_mul(w[:, 0:FSm], al[:, 0:FSm], tt[:, 0:FSm])
            w3 = w[:].rearrange("p (j s) -> p j s", j=nr)
            nc.scalar.dma_start(wv[t], w3[:, :, 0:SM1])

            # weight_total by telescoping: wt = 1 - T[127]; denominator of
            # composite_depth folded as 2*(EPS + wt) = 2*(1 - T[127]) + 2EPS
            wt2 = p_sm.tile([P, nr], f32, tag="wt2")
            nc.scalar.activation(
                wt2[:], tt3[:, :, SM1:S], AF.Identity, scale=-2.0, bias=2.0
            )

            # depths_mid = 0.5*(d[s]+d[s+1]) (output), scaled in place
            nc.scalar.mul(dms[:, 0:FSm], dms[:, 0:FSm], 0.5)
            dms3 = dms[:].rearrange("p (j s) -> p j s", j=nr)
            nc.scalar.dma_start(dmv[t], dms3[:, :, 0:SM1])

            st2[t] = (ct, dt, w, wt2)

        def s3(t):
            """v, products, reductions, composites."""
            ct, dt, w, wt2 = st2.pop(t)
            w3 = w[:].rearrange("p (j s) -> p j s", j=nr)
            cflat = ct[:].rearrange("p (f c) -> p f c", c=3)

            # v[s] = w[s-1] + w[s] with one-sided boundaries: the composite
            # sums become  rgb = sum_s v[s]*c[s] - 1  and
            # depth_num = 0.5 * sum_s v[s]*d[s].  Flat add + boundary fixups.
            v = p_v.tile([P, FS], f32)
            nc.gpsimd.tensor_tensor(v[:, 1:FS], w[:, 0:FSm], w[:, 1:FS], ALU.add)
            v3 = v[:].rearrange("p (j s) -> p j s", j=nr)
            v_fix = v[:].rearrange("p (j s) -> p j s", j=nr)[:, :, 0 : S : SM1]
            w_fix = w[:].rearrange("p (j s) -> p j s", j=nr)[:, :, 0 : SM1 : SM1 - 1]
            nc.scalar.copy(v_fix, w_fix)

            # products + per-ray reductions
            pr_vd = p_scr.tile([P, FS], f32, tag="pr_vd")
            pr_dve = p_scr.tile([P, FS], f32, tag="pr_dve")
            prv3 = pr_vd[:].rearrange("p (j s) -> p j s", j=nr)
            prd3 = pr_dve[:].rearrange("p (j s) -> p j s", j=nr)

            dsum = p_sm.tile([P, nr], f32, tag="dsum")
            nc.gpsimd.tensor_tensor(pr_vd[:], v[:], dt[:], ALU.mult)
            nc.vector.tensor_reduce(
                out=dsum[:], in_=prv3, axis=mybir.AxisListType.X, op=ALU.add
            )

            rgbacc = p_sm.tile([P, nr * 3], f32, tag="rgbacc")
            racc3 = rgbacc[:].rearrange("p (j c) -> p j c", j=nr)
            for ch in (0, 1, 2):
                nc.vector.tensor_tensor(pr_dve[:], v[:], cflat[:, :, ch], ALU.mult)
                nc.vector.tensor_reduce(
                    out=racc3[:, :, ch : ch + 1],
                    in_=prd3,
                    axis=mybir.AxisListType.X,
                    op=ALU.add,
                )
            rgbt = p_sm.tile([P, nr * 3], f32, tag="rgb")
            nc.scalar.activation(rgbt[:], rgbacc[:], AF.Identity, bias=-1.0)
            nc.scalar.dma_start(rgbv[t], rgbt[:])

            # composite_depth = (0.5*dsum) / (EPS + wt), clipped with exact
            # min/max as 1-port tensor_tensor ops (no GPSIMD port contention)
            rec = p_sm.tile([P, nr], f32, tag="rec")
            nc.vector.reciprocal(rec[:], wt2[:])
            dep = p_sm.tile([P, nr], f32, tag="dep")
            nc.vector.tensor_mul(dep[:], dsum[:], rec[:])
            dep2 = p_sm.tile([P, nr], f32, tag="dep2")
            nc.vector.tensor_tensor(dep2[:], dep[:], clip_lo, ALU.max)
            nc.vector.tensor_tensor(dep2[:], dep2[:], clip_hi, ALU.min)
            nc.scalar.dma_start(depv[t], dep2[:])

        s1(0)
        if n_tiles > 1:
            s1(1)
        for t in range(n_tiles):
            if t + 2 < n_tiles:
                s1(t + 2)
            s2(t)
            if t >= 1:
                s3(t - 1)
        s3(n_tiles - 1)

    nc.compile()
    return nc


def _get_program():
    key = (PER_CORE, NR)
    if key not in _PROGRAMS:
        _PROGRAMS[key] = build_program(*key)
    return _PROGRAMS[key]


def kernel(colors, density_logits, depths):
    from concourse.bass_utils import run_bass_kernel_spmd

    colors = np.ascontiguousarray(np.asarray(colors, dtype=np.float32))
    dl = np.ascontiguousarray(np.asarray(density_logits, dtype=np.float32))
    d = np.ascontiguousarray(np.asarray(depths, dtype=np.float32))

    cf = colors.reshape(G, S, 3)
    dlf = dl.reshape(G, S)
    df = d.reshape(G, S)

    nc = _get_program()
    in_maps = [
        {
            "colors": cf[c * PER_CORE : (c + 1) * PER_CORE],
            "density_logits": dlf[c * PER_CORE : (c + 1) * PER_CORE],
            "depths": df[c * PER_CORE : (c + 1) * PER_CORE],
        }
        for c in range(N_CORES)
    ]
    res = run_bass_kernel_spmd(nc, in_maps, list(range(N_CORES))).results

    w = np.concatenate([r["weights"] for r in res]).reshape(B, R, SM1, 1)
    al = np.concatenate([r["alpha"] for r in res]).reshape(B, R, SM1, 1)
    dm = np.concatenate([r["depths_mid"] for r in res]).reshape(B, R, SM1, 1)
    rgb = np.concatenate([r["rgb"] for r in res]).reshape(B, R, 3)
    dep = np.concatenate([r["depth"] for r in res]).reshape(B, R, 1)

    # weights_all is the same array as weights in the reference.
    return (rgb, dep, w, w, al, dm)
